# revision 15
# baseline (speedup 1.0000x reference)
"""Multi-head attention (quirky Dense(d_k) variant) on 8 trn2 NeuronCores.

Sharding: data-parallel over batch (B=2), tensor-parallel over heads
(8 heads -> 4 groups of 2 heads). Core c: batch c//4, head-group c%4.
Each core is fully independent (no collectives); host sums the 4 partial
outputs per batch (Wo row-sharded -> partial sums).

Hardware layout constraints honored here: engine-op partition bases must be
32-aligned, matmul PSUM outputs must start at partition 0, and fp32r matmul
inputs must be produced as fp32r. The two heads' 16 dims sit at partitions
0-15 and 32-47 (padded weights from host).

Per-core dataflow (L=2048, d_model=1024):
  qT/kT/vT = W48^T @ X^T       (PE, f32r, X^T pre-transposed on host)
  v        = transpose(vT)     (PE transpose, interleaved with ones cols)
  s^T tile = k_h^T . q_h       (PE, [128 Lk x 2048 q] per Lk-tile)
  e        = exp(s^T * scale)  (ACT, no max subtraction: |s| < ~1)
  o^T/rsum = [v|1]^T @ e       (PE, accumulated over Lk tiles in PSUM [17, L])
  o^T     /= rsum              (copy to SBUF, rowsum via DRAM-roundtrip
                                broadcast, DVE reciprocal + mul into oT stack)
  y        = [oT;1]^T @ [Wo;bo](PE, bias via ones row 64)
"""

import math
import sys

sys.path.insert(0, "/opt/trn_rl_repo")

import numpy as np

import concourse.bass as bass
import concourse.mybir as mybir
import concourse.tile as tile
from concourse import bacc
from concourse.bass_utils import run_bass_kernel_spmd

H = 8
D_MODEL = 1024
D_K = 128          # projection width (d_model / h)
HD = 16            # per-head dim after reshape
B, L = 2, 2048
DSL = 32           # per-core slice of D_K (2 heads x 16)
DP = 48            # padded: head0 dims at 0-15, head1 at 32-47
CC = 8             # contraction chunks of 128 over d_model
SCALE = 1.0 / math.sqrt(float(D_K))   # reference scales by sqrt(d_k)=sqrt(128)
F32 = mybir.dt.float32
F32R = mybir.dt.float32r  # 4x faster than fp32 on the PE at N>=256

_CACHE = {}


def _build_nc():
    nc = bacc.Bacc(None, target_bir_lowering=False)

    xq = nc.declare_dram_parameter("xq_t", [D_MODEL, L], F32R, isOutput=False)
    xk = nc.declare_dram_parameter("xk_t", [D_MODEL, L], F32R, isOutput=False)
    xv = nc.declare_dram_parameter("xv_t", [D_MODEL, L], F32R, isOutput=False)
    wq = nc.declare_dram_parameter("wq", [D_MODEL, DP], F32R, isOutput=False)
    wk = nc.declare_dram_parameter("wk", [D_MODEL, DP], F32R, isOutput=False)
    wv = nc.declare_dram_parameter("wv", [D_MODEL, DP], F32R, isOutput=False)
    bqkv = nc.declare_dram_parameter("bqkv", [3, DP], F32, isOutput=False)
    # rows 0-15: Wo head0; 32-47: Wo head1; 64: bo (or 0); rest zeros
    wo = nc.declare_dram_parameter("wo", [65, D_MODEL], F32R, isOutput=False)
    identp = nc.declare_dram_parameter("identp", [128, 128], F32R, isOutput=False)
    ones16 = nc.declare_dram_parameter("ones16", [128, 16], F32R, isOutput=False)
    y = nc.declare_dram_parameter("y", [L, D_MODEL], F32, isOutput=True)
    import os
    dbg = os.environ.get("KERNEL_DEBUG", "0") == "1"
    if dbg:
        qt_d = nc.declare_dram_parameter("qt_d", [DP, L], F32R, isOutput=True)
        kt_d = nc.declare_dram_parameter("kt_d", [DP, L], F32R, isOutput=True)
        vt_d = nc.declare_dram_parameter("vt_d", [DP, L], F32R, isOutput=True)
        vsb_d = nc.declare_dram_parameter("vsb_d", [128, 576], F32R, isOutput=True)
        ot_d = nc.declare_dram_parameter("ot_d", [65, L], F32R, isOutput=True)
        cps0_d = nc.declare_dram_parameter("cps0_d", [17, L], F32, isOutput=True)
        cps1_d = nc.declare_dram_parameter("cps1_d", [17, L], F32, isOutput=True)

    Exp = mybir.ActivationFunctionType.Exp
    Ident = mybir.ActivationFunctionType.Identity

    with tile.TileContext(nc) as tc:
        with (
            tc.tile_pool(name="const", bufs=1) as constp,
            tc.tile_pool(name="xin", bufs=3) as xpool,
            tc.tile_pool(name="qk", bufs=1) as qkpool,
            tc.tile_pool(name="ps", bufs=2, space="PSUM") as psA,
            tc.tile_pool(name="pso", bufs=1, space="PSUM") as psO,
            tc.tile_pool(name="ep", bufs=3) as epool,
            tc.tile_pool(name="yp", bufs=2) as ypool,
            tc.tile_pool(name="misc", bufs=2) as misc,
            tc.tile_pool(name="dr", bufs=2, space="DRAM") as drpool,
        ):
            scratch = drpool.tile([2, L], F32)
            # ---- constants ----
            wq_sb = constp.tile([128, CC, DP], F32R)
            nc.sync.dma_start(out=wq_sb, in_=wq[:].rearrange("(c p) d -> p c d", p=128))
            wk_sb = constp.tile([128, CC, DP], F32R)
            nc.sync.dma_start(out=wk_sb, in_=wk[:].rearrange("(c p) d -> p c d", p=128))
            wv_sb = constp.tile([128, CC, DP], F32R)
            nc.sync.dma_start(out=wv_sb, in_=wv[:].rearrange("(c p) d -> p c d", p=128))
            wo_sb = constp.tile([65, D_MODEL], F32R)
            nc.sync.dma_start(out=wo_sb, in_=wo[:])
            bias_sb = constp.tile([DP, 3], F32)
            nc.sync.dma_start(out=bias_sb, in_=bqkv[:].rearrange("b d -> d b"))
            ident = constp.tile([128, 128], F32R)
            nc.sync.dma_start(out=ident, in_=identp[:])

            # v in natural layout, interleaved with ones columns:
            # per Lk-tile t (36 cols): [16 v_h0 | 1 | pad | 16 v_h1 | 1 | pad]
            v_sb = constp.tile([128, 16 * 36], F32R)
            v_sb3i = v_sb.rearrange("p (t s) -> p t s", s=36)
            nc.sync.dma_start(out=v_sb3i[:, :, 16:17], in_=ones16[:].rearrange("p (t o) -> p t o", o=1))
            nc.sync.dma_start(out=v_sb3i[:, :, 34:35], in_=ones16[:].rearrange("p (t o) -> p t o", o=1))

            qT = qkpool.tile([DP, L], F32R)
            kT = qkpool.tile([DP, L], F32R)
            vT = qkpool.tile([64, L], F32R)
            # oT rows: 0-15 = head0 o^T, 32-47 = head1 o^T, 64 = ones, rest 0
            oT = qkpool.tile([65, L], F32R)
            zsrc = qkpool.tile([65, L], F32)
            nc.vector.memset(zsrc, 0.0)
            nc.vector.memset(zsrc[64:65, :], 1.0)
            nc.vector.tensor_copy(oT, zsrc)
            # zero vT rows 48-63 before projections fill rows 0-47 (the
            # 64-partition transpose reads all 64 rows)
            nc.vector.tensor_copy(vT[32:64, :], zsrc[0:32, :])

            # ---- projections: outT = W48^T @ X^T (accumulate over c-chunks) ----
            for x_dram, w_sb, bcol, outT in (
                (xq, wq_sb, 0, qT),
                (xk, wk_sb, 1, kT),
                (xv, wv_sb, 2, vT),
            ):
                ph = [psA.tile([DP, 1024], F32, tag="ps", name=f"ph{_i}") for _i in range(2)]
                for cc in range(CC):
                    xt = xpool.tile([128, L], F32R, tag="x")
                    nc.sync.dma_start(out=xt, in_=x_dram[cc * 128:(cc + 1) * 128, :])
                    for half in (0, 1):
                        for sub in (0, 1):
                            c0 = half * 1024 + sub * 512
                            nc.tensor.matmul(
                                ph[half][:, sub * 512:(sub + 1) * 512],
                                lhsT=w_sb[:, cc, :],
                                rhs=xt[:, c0:c0 + 512],
                                start=(cc == 0),
                                stop=(cc == CC - 1),
                            )
                for half in (0, 1):
                    nc.scalar.activation(
                        outT[0:DP, half * 1024:(half + 1) * 1024],
                        ph[half],
                        Ident,
                        bias=bias_sb[:, bcol:bcol + 1],
                    )

            # ---- transpose v: vT [64, 2048] -> v_sb interleaved ----
            # (64-partition transpose: 48-partition transposes corrupt rows 32+)
            pvt = psA.tile([128, 16 * 64], F32R, tag="ps")
            for t in range(16):
                nc.tensor.transpose(
                    pvt[:, t * 64:(t + 1) * 64],
                    vT[:, t * 128:(t + 1) * 128],
                    ident[0:64, 0:64],
                )
            v_sb3 = v_sb.rearrange("p (t s) -> p t s", s=36)
            pvt3 = pvt.rearrange("p (t s) -> p t s", s=64)
            for h in (0, 1):
                nc.vector.tensor_copy(
                    v_sb3[:, :, 18 * h:18 * h + 16], pvt3[:, :, 32 * h:32 * h + 16]
                )

            # ---- attention per head: s^T tiles -> exp -> AV accumulate ----
            for h in (0, 1):
                psoo = psO.tile([17, L], F32, tag="oo")  # rows 0-15 o^T, 16 rowsum
                for t in range(16):
                    kslice = kT[32 * h:32 * h + 16, t * 128:(t + 1) * 128]
                    for qh in (0, 1):
                        ps_s = psA.tile([128, 1024], F32, tag="ps")
                        for sub in (0, 1):
                            q0 = qh * 1024 + sub * 512
                            nc.tensor.matmul(
                                ps_s[:, sub * 512:(sub + 1) * 512],
                                lhsT=kslice,
                                rhs=qT[32 * h:32 * h + 16, q0:q0 + 512],
                                start=True,
                                stop=True,
                            )
                        et = epool.tile([128, 1024], F32R, tag="e")
                        nc.scalar.activation(et, ps_s, Exp, scale=SCALE)
                        for sub in (0, 1):
                            q0 = qh * 1024 + sub * 512
                            nc.tensor.matmul(
                                psoo[:, q0:q0 + 512],
                                lhsT=v_sb[:, t * 36 + 18 * h:t * 36 + 18 * h + 17],
                                rhs=et[:, sub * 512:(sub + 1) * 512],
                                start=(t == 0),
                                stop=(t == 15),
                            )
                # normalize: o^T /= rowsum
                cps = misc.tile([17, L], F32, tag="cps")
                nc.vector.tensor_copy(cps, psoo)
                nc.sync.dma_start(out=scratch[h:h+1, :], in_=cps[16:17, :])
                rb = misc.tile([16, L], F32, tag="rb")
                nc.sync.dma_start(
                    out=rb,
                    in_=scratch[h:h + 1, :].to_broadcast((16, L)),
                )
                rb2 = misc.tile([16, L], F32, tag="rb2")
                nc.vector.reciprocal(rb2, rb)
                nc.vector.tensor_mul(oT[32 * h:32 * h + 16, :], cps[0:16, :], rb2)
                if dbg:
                    nc.sync.dma_start(out=(cps0_d if h == 0 else cps1_d)[:], in_=cps)

            if dbg:
                nc.sync.dma_start(out=qt_d[:], in_=qT)
                nc.sync.dma_start(out=kt_d[:], in_=kT)
                nc.sync.dma_start(out=vt_d[:], in_=vT[0:DP, :])
                nc.sync.dma_start(out=vsb_d[:], in_=v_sb)
                nc.sync.dma_start(out=ot_d[:], in_=oT)

            # ---- output projection: y = [oT;1]^T @ [Wo;bo] ----
            for i in range(16):
                py_ = psA.tile([128, 1024], F32, tag="ps")
                for sub in (0, 1):
                    nc.tensor.matmul(
                        py_[:, sub * 512:(sub + 1) * 512],
                        lhsT=oT[:, i * 128:(i + 1) * 128],
                        rhs=wo_sb[:, sub * 512:(sub + 1) * 512],
                        start=True,
                        stop=True,
                    )
                yt = ypool.tile([128, 1024], F32, tag="y")
                nc.vector.tensor_copy(yt, py_)
                nc.sync.dma_start(out=y[i * 128:(i + 1) * 128, :], in_=yt)

    nc.finalize()
    return nc


def _get_nc():
    if "nc" not in _CACHE:
        _CACHE["nc"] = _build_nc()
    return _CACHE["nc"]


def _pad48(w32):
    # [*, 32] -> [*, 48] with head0 dims at 0-15, head1 at 32-47
    out = np.zeros(w32.shape[:-1] + (DP,), np.float32)
    out[..., 0:16] = w32[..., 0:16]
    out[..., 32:48] = w32[..., 16:32]
    return out


def make_in_maps(queries, keys, values, Wq, bq, Wk, bk, Wv, bv, Wo, bo):
    xqt = [np.ascontiguousarray(queries[b].T) for b in range(B)]
    xkt = [np.ascontiguousarray(keys[b].T) for b in range(B)]
    xvt = [np.ascontiguousarray(values[b].T) for b in range(B)]

    in_maps = []
    for core in range(8):
        b, hg = core // 4, core % 4
        s = DSL * hg
        wo65 = np.zeros((65, D_MODEL), np.float32)
        wo65[0:16] = Wo[s:s + 16]
        wo65[32:48] = Wo[s + 16:s + 32]
        if hg == 0:
            wo65[64] = bo
        in_maps.append({
            "xq_t": xqt[b],
            "xk_t": xkt[b],
            "xv_t": xvt[b],
            "wq": _pad48(Wq[:, s:s + DSL]),
            "wk": _pad48(Wk[:, s:s + DSL]),
            "wv": _pad48(Wv[:, s:s + DSL]),
            "bqkv": _pad48(
                np.stack([bq[s:s + DSL], bk[s:s + DSL], bv[s:s + DSL]])
            ),
            "wo": wo65,
            "identp": np.eye(128, dtype=np.float32),
            "ones16": np.ones((128, 16), np.float32),
        })
    return in_maps


def kernel(queries, keys, values, Wq, bq, Wk, bk, Wv, bv, Wo, bo, **_unused):
    queries = np.asarray(queries, dtype=np.float32)
    keys = np.asarray(keys, dtype=np.float32)
    values = np.asarray(values, dtype=np.float32)
    Wq, Wk, Wv = (np.asarray(a, dtype=np.float32) for a in (Wq, Wk, Wv))
    Wo = np.asarray(Wo, dtype=np.float32)
    bq, bk, bv, bo = (np.asarray(a, dtype=np.float32) for a in (bq, bk, bv, bo))

    nc = _get_nc()
    in_maps = make_in_maps(queries, keys, values, Wq, bq, Wk, bk, Wv, bv, Wo, bo)
    res = run_bass_kernel_spmd(nc, in_maps, core_ids=list(range(8)))
    out = np.zeros((B, L, D_MODEL), np.float32)
    for core in range(8):
        out[core // 4] += res.results[core]["y"]
    return out
